# revision 1
# baseline (speedup 1.0000x reference)
"""Trainium2 Bass kernel for 4D conv (VALID, stride 1, channels-last).

x: [4, 20, 20, 40, 40, 8] f32, kernel: [3, 3, 3, 3, 8, 16], bias: [...,16]
out: [4, 18, 18, 38, 38, 16] f32

Strategy (8 NeuronCores, SPMD):
  - Shard (batch 4) x (T-halves 2) -> 8 shards. Each core gets
    x[b, 9*th : 9*th+11] (halo 2 in T) and computes out[b, 9*th : 9*th+9].
  - On-chip: per T-plane, load natural (z,h)-row-major chunks, PE-transpose
    to (w,c)-major tiles X_T[(w-wstart)*8+c, z*40+h] for 5 overlapping
    w-blocks (w starts 0,8,16,24,32).
  - Conv as Toeplitz-banded matmul: lhsT[(wl,c), (w'l,co)] holds
    k[dt,dz,dh, wl-w'l, c, co] (band 0<=wl-w'l<=2), contracting K=(10w x 8c)
    =80 rows; M=128=(8 w' x 16 co); 27 taps (dt,dz,dh) accumulate in PSUM
    with free-dim shifts dz*40+dh into X_T columns; N=342=(9 z' x 38 h').
  - Bias added during PSUM->SBUF evacuation (per-partition scalar), output
    written in a blocked layout [t',zh,wb,128,342]; host rearranges.
"""

import sys

if "/opt/trn_rl_repo" not in sys.path:
    sys.path.insert(0, "/opt/trn_rl_repo")

from contextlib import ExitStack

import ml_dtypes
import numpy as np

import concourse.bass as bass
import concourse.tile as tile
from concourse import bacc, mybir
from concourse.bass_utils import run_bass_kernel_spmd
from concourse.masks import make_identity

F32 = mybir.dt.float32
BF16 = mybir.dt.bfloat16

# Problem geometry (hardcoded)
B, T, Z, H, W, CIN = 4, 20, 20, 40, 40, 8
KT = KZ = KH = KW = 3
COUT = 16
TP = 9  # output t' per core (T' = 18 split across 2 cores)
TL = TP + KT - 1  # input t planes per core = 11
ZP, HP, WP = Z - 2, H - 2, W - 2  # 18, 38, 38
NTAP = KT * KZ * KH  # 27 taps accumulated in PSUM
WBLOCKS = 5  # w starts 0,8,16,24,32
NCOLS = 9 * HP  # 342 columns per matmul (9 z' x 38 h')

# transpose free-slices of the (w,c)=320 row: (offset, width) per w-block
_TR_SLICES = [(0, 128), (64, 128), (128, 128), (192, 128), (256, 64)]

LAST_RESULTS = None  # BassKernelResults of the most recent run (for test.py)
REPS = 1  # >1 wraps the body in a hardware loop (timing experiments only)


def _build_program():
    nc = bacc.Bacc("TRN2", target_bir_lowering=False, debug=False, num_devices=8)

    x_d = nc.dram_tensor("x", [TL, Z, H, W, CIN], F32, kind="ExternalInput").ap()
    wt_d = nc.dram_tensor("wt", [NTAP, 128, 128], BF16, kind="ExternalInput").ap()
    bias_d = nc.dram_tensor("bias128", [128, 1], F32, kind="ExternalInput").ap()
    out_d = nc.dram_tensor(
        "out", [TP, 2, WBLOCKS, 128, NCOLS], F32, kind="ExternalOutput"
    ).ap()

    with ExitStack() as ctx:
        tc = ctx.enter_context(tile.TileContext(nc))
        consts = ctx.enter_context(tc.tile_pool(name="consts", bufs=1))
        xt_pool = ctx.enter_context(tc.tile_pool(name="xt", bufs=4))
        chunk_pool = ctx.enter_context(tc.tile_pool(name="chunk", bufs=3))
        tpsum_pool = ctx.enter_context(tc.tile_pool(name="tpsum", bufs=3, space="PSUM"))
        mpsum_pool = ctx.enter_context(tc.tile_pool(name="mpsum", bufs=4, space="PSUM"))
        outp_pool = ctx.enter_context(tc.tile_pool(name="outp", bufs=4))

        ident = consts.tile([128, 128], F32)
        make_identity(nc, ident)

        wt_s = consts.tile([128, NTAP * 128], BF16)
        for j in range(NTAP):
            nc.sync.dma_start(wt_s[:, j * 128 : (j + 1) * 128], wt_d[j])

        bias_s = consts.tile([128, 1], F32)
        nc.sync.dma_start(bias_s[:, :], bias_d[:, :])

        planes = {}

        def prep_plane(t):
            tiles = [
                xt_pool.tile([128, Z * H], BF16, tag=f"xt{i}", name=f"xt{i}_{t}")
                for i in range(WBLOCKS)
            ]
            x_t = x_d[t].rearrange("z h w c -> (z h) (w c)")  # [800, 320]
            nrows = Z * H
            for r0 in range(0, nrows, 128):
                r = min(128, nrows - r0)
                ch = chunk_pool.tile([128, W * CIN], F32, tag="chunk")
                nc.sync.dma_start(ch[:r, :], x_t[r0 : r0 + r, :])
                for bi, (f0, fw) in enumerate(_TR_SLICES):
                    ps = tpsum_pool.tile([128, 128], F32, tag="tps")
                    nc.tensor.transpose(ps[:fw, :r], ch[:r, f0 : f0 + fw], ident[:r, :r])
                    # f32 PSUM -> bf16 SBUF cast during evacuation
                    nc.vector.tensor_copy(tiles[bi][:fw, r0 : r0 + r], ps[:fw, :r])
            planes[t] = tiles

        def do_tp(tp):
            for zh in range(2):
                for wb in range(WBLOCKS):
                    k = 64 if wb == WBLOCKS - 1 else 80
                    ps = mpsum_pool.tile([128, NCOLS], F32, tag="mps")
                    for j in range(NTAP):
                        dt_, r = divmod(j, KZ * KH)
                        dz, dh = divmod(r, KH)
                        v = planes[tp + dt_][wb].rearrange("p (z h) -> p z h", h=H)
                        rhs = v[0:k, zh * 9 + dz : zh * 9 + dz + 9, dh : dh + HP]
                        lhsT = wt_s[0:k, j * 128 : (j + 1) * 128]
                        nc.tensor.matmul(
                            ps[:, :], lhsT, rhs, start=(j == 0), stop=(j == NTAP - 1)
                        )
                    ot = outp_pool.tile([128, NCOLS], F32, tag="out")
                    nc.vector.tensor_scalar_add(ot[:, :], ps[:, :], bias_s[:, 0:1])
                    nc.sync.dma_start(out_d[tp, zh, wb], ot[:, :])

        def body():
            planes.clear()
            for t in range(KT):
                prep_plane(t)
            for tp in range(TP):
                do_tp(tp)
                if tp + KT < TL:
                    prep_plane(tp + KT)

        if REPS > 1:
            with tc.For_i(0, REPS, 1):
                body()
        else:
            body()

    nc.compile()
    return nc


def _host_weights(kern):
    """Toeplitz-banded weight matrices, one per (dt,dz,dh) tap."""
    wt = np.zeros((NTAP, 128, 128), np.float32)
    for dt_ in range(KT):
        for dz in range(KZ):
            for dh in range(KH):
                j = (dt_ * KZ + dz) * KH + dh
                for dw in range(KW):
                    for wpl in range(8):
                        wl = wpl + dw
                        wt[
                            j,
                            wl * CIN : (wl + 1) * CIN,
                            wpl * COUT : (wpl + 1) * COUT,
                        ] = kern[dt_, dz, dh, dw]
    return wt.astype(ml_dtypes.bfloat16)


def kernel(x, kernel, bias):
    global LAST_RESULTS
    x = np.asarray(x, np.float32)
    kern = np.asarray(kernel, np.float32)
    bias = np.asarray(bias, np.float32).reshape(COUT)

    wt = _host_weights(kern)
    bias128 = np.tile(bias, 8).reshape(128, 1).astype(np.float32)

    nc = _build_program()

    core_ids = list(range(8))
    in_maps = []
    for core in core_ids:
        b, th = divmod(core, 2)
        in_maps.append(
            {
                "x": np.ascontiguousarray(x[b, 9 * th : 9 * th + TL]),
                "wt": wt,
                "bias128": bias128,
            }
        )

    res = run_bass_kernel_spmd(nc, in_maps, core_ids)
    LAST_RESULTS = res

    out = np.empty((B, 2 * TP, ZP, HP, WP, COUT), np.float32)
    for core in core_ids:
        b, th = divmod(core, 2)
        a = res.results[core]["out"].reshape(TP, 2, WBLOCKS, 8, COUT, 9, HP)
        a = a.transpose(0, 1, 5, 6, 2, 3, 4).reshape(TP, ZP, HP, 40, COUT)
        out[b, 9 * th : 9 * th + TP] = a[:, :, :, :WP, :]
    return out



# revision 2
# speedup vs baseline: 22.9454x; 22.9454x over previous
"""Trainium2 Bass kernel for 4D conv (VALID, stride 1, channels-last).

x: [4, 20, 20, 40, 40, 8] f32, kernel: [3, 3, 3, 3, 8, 16], bias: [...,16]
out: [4, 18, 18, 38, 38, 16] f32

Strategy (8 NeuronCores, SPMD):
  - Shard (batch 4) x (T-halves 2) -> 8 shards; halo 2 in T per shard.
  - Host pre-packs each shard into a 2D-Toeplitz im2col layout
    x2[t, (hl,wl,c), (z, hb, wb)] bf16 with 2x2 output blocks, so
    K=128=(4*4*8) and M=64=(2h'x2w'x16co).
  - Conv = 9 accumulation passes (dt,dz) per PSUM chunk; the dz shift is a
    pure column offset (dz*361); dt picks the plane.
  - 6498 output columns per t' split into 7 balanced pairs of (464|464)
    (last 465|465); pair halves run on PE column-groups via tile_position
    (0,0)/(0,64) concurrently, sharing one PSUM bank.
  - Bias added during PSUM->SBUF evacuation (bf16 out, halves DMA);
    host rearranges the blocked output and upcasts.
"""

import sys

if "/opt/trn_rl_repo" not in sys.path:
    sys.path.insert(0, "/opt/trn_rl_repo")

from contextlib import ExitStack

import ml_dtypes
import numpy as np

import concourse.bass as bass
import concourse.tile as tile
from concourse import bacc, mybir
from concourse.bass_utils import run_bass_kernel_spmd

F32 = mybir.dt.float32
BF16 = mybir.dt.bfloat16

# Problem geometry (hardcoded)
B, T, Z, H, W, CIN = 4, 20, 20, 40, 40, 8
KT = KZ = KH = KW = 3
COUT = 16
TP = 9  # output t' per core (T' = 18 split across 2 cores)
TL = TP + KT - 1  # input t planes per core = 11
ZP, HP, WP = Z - 2, H - 2, W - 2  # 18, 38, 38
HB = WB = 19  # 2x2 output blocks: 38 = 2*19
NZCOL = HB * WB  # 361 columns per z'
NCOL = ZP * NZCOL  # 6498 output columns per t'
NPASS = KT * KZ  # 9 accumulation passes (dt, dz)
NPAIR = 7  # balanced pairs: 6x(464|464) + 1x(465|465)
PAIRW = [(464, 464)] * 6 + [(465, 465)]
PAIROFF = [928 * p for p in range(6)] + [5568]
NSPLIT = 2  # plane DMA split for queue parallelism

LAST_RESULTS = None
REPS = 1
TIMING = False  # True: all big I/O becomes Internal (garbage data, tiny transfer)


def _build_program():
    nc = bacc.Bacc("TRN2", target_bir_lowering=False, debug=False, num_devices=8)

    big = "Internal" if TIMING else "ExternalInput"
    x2_d = nc.dram_tensor("x2", [TL, 128, Z * NZCOL], BF16, kind=big).ap()
    wt_d = nc.dram_tensor("wt", [NPASS, 128, 64], BF16, kind=big).ap()
    bias_d = nc.dram_tensor("bias128", [128, 1], F32, kind="ExternalInput").ap()
    out_d = nc.dram_tensor(
        "out",
        [TP, NPAIR, 128, 512],
        BF16,
        kind="Internal" if TIMING else "ExternalOutput",
    ).ap()
    small_d = (
        nc.dram_tensor("small", [128, 1], F32, kind="ExternalOutput").ap()
        if TIMING
        else None
    )

    with ExitStack() as ctx:
        tc = ctx.enter_context(tile.TileContext(nc))
        consts = ctx.enter_context(tc.tile_pool(name="consts", bufs=1))
        xt_pool = ctx.enter_context(tc.tile_pool(name="xt", bufs=4))
        mpsum_pool = ctx.enter_context(tc.tile_pool(name="mpsum", bufs=4, space="PSUM"))
        outp_pool = ctx.enter_context(tc.tile_pool(name="outp", bufs=4))

        wt_s = consts.tile([128, NPASS * 64], BF16)
        for j in range(NPASS):
            nc.sync.dma_start(wt_s[:, j * 64 : (j + 1) * 64], wt_d[j])

        bias_s = consts.tile([128, 1], F32)
        nc.sync.dma_start(bias_s[:, :], bias_d[:, :])

        planes = {}
        ncols = Z * NZCOL

        def prep_plane(t):
            xt = xt_pool.tile([128, ncols], BF16, tag="x2", name=f"x2_{t}")
            step = (ncols + NSPLIT - 1) // NSPLIT
            for s in range(NSPLIT):
                c0 = s * step
                c1 = min(ncols, c0 + step)
                nc.sync.dma_start(xt[:, c0:c1], x2_d[t, :, c0:c1])
            planes[t] = xt

        def do_tp(tp):
            for pair in range(NPAIR):
                na, nb = PAIRW[pair]
                poff = PAIROFF[pair]
                ps = mpsum_pool.tile([128, 512], F32, tag="mps")
                for j in range(NPASS):
                    dt_, dz = divmod(j, KZ)
                    v = planes[tp + dt_]
                    lhsT = wt_s[:, j * 64 : (j + 1) * 64]
                    offa = dz * NZCOL + poff
                    st, sp = j == 0, j == NPASS - 1
                    nc.tensor.matmul(
                        ps[0:64, 0:na],
                        lhsT,
                        v[:, offa : offa + na],
                        start=st,
                        stop=sp,
                        tile_position=(0, 0),
                    )
                    nc.tensor.matmul(
                        ps[64:128, 0:nb],
                        lhsT,
                        v[:, offa + na : offa + na + nb],
                        start=st,
                        stop=sp,
                        tile_position=(0, 64),
                    )
                ot = outp_pool.tile([128, 512], BF16, tag="out")
                nc.vector.tensor_scalar_add(
                    ot[:, 0:na], ps[:, 0:na], bias_s[:, 0:1]
                )
                nc.sync.dma_start(out_d[tp, pair], ot[:, :])

        def body():
            planes.clear()
            for t in range(KT):
                prep_plane(t)
            for tp in range(TP):
                do_tp(tp)
                if tp + KT < TL:
                    prep_plane(tp + KT)

        if REPS > 1:
            with tc.For_i(0, REPS, 1):
                body()
        else:
            body()

        if TIMING:
            nc.sync.dma_start(small_d[:, :], bias_s[:, :])

    nc.compile()
    return nc


def _host_weights(kern):
    """2D-Toeplitz weight matrices, one per (dt,dz) pass: [128 K, 64 M]."""
    wt = np.zeros((NPASS, 4, 4, CIN, 2, 2, COUT), np.float32)
    for j in range(NPASS):
        dt_, dz = divmod(j, KZ)
        for hl in range(4):
            for wl in range(4):
                for hp in range(2):
                    for wp in range(2):
                        dh, dw = hl - hp, wl - wp
                        if 0 <= dh < KH and 0 <= dw < KW:
                            wt[j, hl, wl, :, hp, wp, :] = kern[dt_, dz, dh, dw]
    return wt.reshape(NPASS, 128, 64).astype(ml_dtypes.bfloat16)


def _host_x2(xs):
    """Shard [TL, Z, H, W, CIN] f32 -> [TL, 128, Z*361] bf16 Toeplitz layout."""
    x2 = np.empty((TL, 4, 4, CIN, Z, HB, WB), np.float32)
    for hl in range(4):
        for wl in range(4):
            seg = xs[:, :, hl : hl + 2 * HB - 1 : 2, wl : wl + 2 * WB - 1 : 2, :]
            x2[:, hl, wl] = seg.transpose(0, 4, 1, 2, 3)
    return np.ascontiguousarray(
        x2.reshape(TL, 128, Z * NZCOL).astype(ml_dtypes.bfloat16)
    )


def build_maps(x, kern, bias):
    wt = _host_weights(kern)
    bias128 = np.tile(bias.reshape(COUT), 8).reshape(128, 1).astype(np.float32)
    in_maps = []
    for core in range(8):
        b, th = divmod(core, 2)
        in_maps.append(
            {
                "x2": _host_x2(x[b, 9 * th : 9 * th + TL]),
                "wt": wt,
                "bias128": bias128,
            }
        )
    return in_maps


def _unshard(res_out):
    """[TP, NPAIR, 128, 512] bf16 -> [TP, ZP, HP, WP, COUT] f32."""
    a = np.asarray(res_out).astype(np.float32)
    lin = np.empty((TP, 64, NCOL), np.float32)
    for p in range(NPAIR):
        na, nb = PAIRW[p]
        off = PAIROFF[p]
        lin[:, :, off : off + na] = a[:, p, 0:64, 0:na]
        lin[:, :, off + na : off + na + nb] = a[:, p, 64:128, 0:nb]
    o = lin.reshape(TP, 2, 2, COUT, ZP, HB, WB)
    # [tp, hp, wp, co, z', hb, wb] -> [tp, z', hb, hp, wb, wp, co]
    o = o.transpose(0, 4, 5, 1, 6, 2, 3)
    return o.reshape(TP, ZP, HP, WP, COUT)


def kernel(x, kernel, bias):
    global LAST_RESULTS
    x = np.asarray(x, np.float32)
    kern = np.asarray(kernel, np.float32)
    bias = np.asarray(bias, np.float32)

    nc = _build_program()
    in_maps = build_maps(x, kern, bias)
    res = run_bass_kernel_spmd(nc, in_maps, list(range(8)))
    LAST_RESULTS = res

    out = np.empty((B, 2 * TP, ZP, HP, WP, COUT), np.float32)
    for core in range(8):
        b, th = divmod(core, 2)
        out[b, 9 * th : 9 * th + TP] = _unshard(res.results[core]["out"])
    return out
